# revision 53
# baseline (speedup 1.0000x reference)
"""Trainium2 Bass kernel for nn_ContrastiveLoss_66030827208766.

Strategy (data-parallel over images, captions replicated):
  - 8 cores, 16 images each.  Images are assigned to cores by GLOBAL
    length rank (core = rank % 8, slot = rank // 8), so every core's
    slot-k image has nearly the same valid-object count.  Only valid
    objects are shipped, padded per slot-group to a shared width: group
    A = slots 0-7 padded to Wa = len_sorted[63], group B = slots 8-15
    padded to Wb = len_sorted[127].  One program serves all cores.
  - Padding replicates object 0 (always valid), so a plain max over the
    padded block equals the masked max over valid objects.
  - All matmul operands are bf16 (PE accumulates fp32; end-to-end loss
    error ~1e-5).  Captions are replicated to every core in D-major
    layout [D, w*128 + c]: each 128-column slice is one caption word
    across all 128 captions.
  - Device per core: per caption word w, one matmul (stationary caption
    chunk [D,128], moving packed image-objects [D, C]) -> PSUM bank;
    grouped strided reduce_max over each slot's object block ->
    buf[c, w, slot]; reduce_sum over w; scale by 1/caption_len ->
    two [128 caps, 16 slots] tiles (parts sort by different keys) ->
    DRAM.
  - Host: unpermute slots of each part, add, then the (tiny) triplet
    margin loss reduction in numpy.

Codegen constraint: every TPB instruction can carry at most ONE sync
wait.  Three tactics keep us within it: (1) freshly-DMA'd tiles are
first touched by degenerate 1x1 "junk" matmuls so the real matmuls'
DMA-queue requirements are already observed by the PE; (2) buffers are
laid out so each writer hits a disjoint range (no spurious WAW chains);
(3) a post-pass strips waits that are redundant by construction
(same-engine in-order completion, per-queue DMA FIFO, barrier-covered
drain waits).
"""

import ml_dtypes
import numpy as np

import concourse.bass as bass
import concourse.mybir as mybir
from concourse import tile
from concourse.bass_utils import run_bass_kernel_spmd
from concourse.tile_rust import add_dep_helper

B = 128          # batch (images == captions)
O1, W1 = 36, 50  # part 1: im objects, s words
O2, W2 = 25, 30  # part 2: pred objects, c_r words
D = 128
NCORES = 8
IPC = B // NCORES  # images (slots) per core
G = IPC // 2       # slots per width-group
MARGIN = 0.2
F32 = mybir.dt.float32
BF16 = mybir.dt.bfloat16

LAST_RESULT = None   # BassKernelResults of the most recent run (for test.py)
_NC = None           # cached program
_NC_KEY = None       # widths the cached program was built for


def _build_part(nc, pending, hoist, cap, imt, buf, ps_tiles, ps_cols,
                W, Wa, Wb, cap_piece_cols, cap_key):
    """Emit matmul + grouped-reduce stream for one t2i part.

    Chunk layout: if C = 8*(Wa+Wb) fits one PSUM bank, chunk j of a tile
    sits in bank j (group A at +0, group B at +8*Wa); otherwise each
    chunk takes two banks (A at +0, B at +512).
    """
    C = G * (Wa + Wb)
    if C <= 512:
        banks_per_chunk, offA, offB = 1, 0, G * Wa
    else:
        banks_per_chunk, offA, offB = 2, 0, 512
    wc_per_piece = cap_piece_cols // B

    w = 0
    t_idx = 0
    while w < W:
        ps = ps_tiles[t_idx % len(ps_tiles)]
        cap_chunks = ps_cols[t_idx % len(ps_tiles)] // (512 * banks_per_chunk)
        n = min(cap_chunks, W - w)
        t_idx += 1
        for j in range(n):
            pc = (w + j) // wc_per_piece
            hoist((cap_key, pc),
                  cap[:1, pc * cap_piece_cols:pc * cap_piece_cols + 1])
            cs = cap[:, (w + j) * B:(w + j + 1) * B]
            base = j * banks_per_chunk * 512
            if banks_per_chunk == 1:
                mm = nc.tensor.matmul(ps[:, base:base + C], cs, imt[:],
                                      start=True, stop=True)
                while pending:
                    add_dep_helper(mm.ins, pending.pop().ins, sync=False,
                                   reason="order matmul after wait-carrier")
            else:
                for off, w0, wid in ((offA, 0, Wa), (offB, G * Wa, Wb)):
                    mm = nc.tensor.matmul(
                        ps[:, base + off:base + off + G * wid], cs,
                        imt[:, w0:w0 + G * wid], start=True, stop=True)
                    while pending:
                        add_dep_helper(mm.ins, pending.pop().ins, sync=False,
                                       reason="order matmul after wait-carrier")
        # Two grouped reduces (uniform width within each) covering all n
        # chunks of this tile.
        stride = banks_per_chunk * 512
        v = ps[:, :n * stride].rearrange("p (c x) -> p c x", c=n)
        for off, wid, s0, s1 in ((offA, Wa, 0, G), (offB, Wb, G, IPC)):
            nc.vector.reduce_max(
                buf[:, w:w + n, s0:s1],
                v[:, :, off:off + G * wid].rearrange(
                    "p c (g o) -> p c g o", o=wid),
                axis=mybir.AxisListType.X,
            )
        w += n


def _build_nc(widths):
    (Wa1, Wb1, Wa2, Wb2) = widths
    nc = bass.Bass()
    C1 = G * (Wa1 + Wb1)
    C2 = G * (Wa2 + Wb2)
    capT1 = nc.dram_tensor("capT1", [D, B * W1], BF16, kind="ExternalInput")
    capT2 = nc.dram_tensor("capT2", [D, B * W2], BF16, kind="ExternalInput")
    imT1 = nc.dram_tensor("imT1", [D, C1], BF16, kind="ExternalInput")
    imT2 = nc.dram_tensor("imT2", [D, C2], BF16, kind="ExternalInput")
    rblob = nc.dram_tensor("rblob", [B, 2], F32, kind="ExternalInput")
    out_t = nc.dram_tensor("scores_t", [B, 2 * IPC], F32,
                           kind="ExternalOutput")

    with tile.TileContext(nc) as tc:
        with (
            tc.tile_pool(name="const", bufs=1) as cpool,
            tc.tile_pool(name="psum", bufs=1, space="PSUM") as pspool,
            tc.tile_pool(name="work", bufs=1) as wpool,
        ):
            # ---- input DMAs: 16 total, alternating the two HWDGE rings
            # (even index -> sync, odd -> scalar).  DMAHW bookkeeping
            # lanes are assigned by global round-robin, so each of the 8
            # lanes sees a single issuing engine -> per-lane FIFO holds
            # and own-lane waits are strippable.  The output DMA is
            # emission #16 -> lane 0 (sync), same engine as lane 0's
            # inputs.
            dma_idx = [0]

            def load(dst_ap, src_ap):
                eng = nc.sync if dma_idx[0] % 2 == 0 else nc.scalar
                dma_idx[0] += 1
                return eng.dma_start(dst_ap, src_ap)

            NP1 = 10  # cap1 pieces (5 w-chunks each): early words land early
            NP2 = 1   # cap2: one DMA, only needed after part 1 finishes
            P1C = B * W1 // NP1
            P2C = B * W2 // NP2

            imt1 = cpool.tile([D, C1], BF16, tag="imt1")
            load(imt1[:], imT1[:])
            cap1 = cpool.tile([D, B * W1], BF16, tag="cap1")
            for j in range(NP1):
                load(cap1[:, j * P1C:(j + 1) * P1C],
                     capT1[:, j * P1C:(j + 1) * P1C])
            imt2 = cpool.tile([D, C2], BF16, tag="imt2")
            load(imt2[:], imT2[:])
            cap2 = cpool.tile([D, B * W2], BF16, tag="cap2")
            for j in range(NP2):
                load(cap2[:, j * P2C:(j + 1) * P2C],
                     capT2[:, j * P2C:(j + 1) * P2C])
            rblob_sb = cpool.tile([B, 2], F32, tag="rblob")
            load(rblob_sb[:], rblob[:])
            r1 = rblob_sb[:, 0:1]
            r2 = rblob_sb[:, 1:2]
            # 14 input DMAs; the output DMA is emission #14 -> lane 6,
            # whose earlier user (emission #6) is also sync-issued.
            assert dma_idx[0] == 14, dma_idx

            # w-major: each reduce writes a disjoint contiguous-ish range.
            buf1 = wpool.tile([B, W1, IPC], F32, tag="buf1")
            buf2 = wpool.tile([B, W2, IPC], F32, tag="buf2")

            # Static PSUM: 4-bank + 3-bank ping-pong tiles shared by both
            # parts, 1 junk bank.  (Pool slot rotation would bundle both
            # accessor engines' release waits onto one matmul.)
            psA = pspool.tile([B, 1536], F32, tag="psA", name="psA")
            psB = pspool.tile([B, 1536], F32, tag="psB", name="psB")
            psC = pspool.tile([B, 512], F32, tag="psC", name="psC")
            junk_ps = pspool.tile([1, 1], F32, tag="junk_ps", name="junk_ps")

            hoisted = {}
            pending = []

            def hoist(key, corner_ap):
                if key in hoisted:
                    return
                hoisted[key] = nc.tensor.matmul(
                    junk_ps[:, :], corner_ap, corner_ap,
                    start=True, stop=True, skip_group_check=True,
                )
                pending.append(hoisted[key])

            sout = wpool.tile([B, 2 * IPC], F32, tag="sout")
            s1 = wpool.tile([B, IPC], F32, tag="s1")
            s2 = wpool.tile([B, IPC], F32, tag="s2")

            hoist(("imt1",), imt1[:1, :1])
            _build_part(nc, pending, hoist, cap1, imt1, buf1, [psA, psB, psC],
                        [1536, 1536, 512], W1, Wa1, Wb1, P1C, "cap1")
            # Part-1 epilogue emitted before part 2: the DVE executes its
            # queue in order, so this overlaps part-2 matmuls.
            nc.vector.reduce_sum(s1[:], buf1[:].rearrange("p w i -> p i w"),
                                 axis=mybir.AxisListType.X)
            nc.vector.tensor_scalar_mul(sout[:, :IPC], s1[:], r1)

            hoist(("imt2",), imt2[:1, :1])
            _build_part(nc, pending, hoist, cap2, imt2, buf2, [psA, psB, psC],
                        [1536, 1536, 512], W2, Wa2, Wb2, P2C, "cap2")
            nc.vector.reduce_sum(s2[:], buf2[:].rearrange("p w i -> p i w"),
                                 axis=mybir.AxisListType.X)
            nc.vector.tensor_scalar_mul(sout[:, IPC:], s2[:], r2)
            out_dma = nc.sync.dma_start(out_t[:], sout[:])

    # ---- wait-strip post-pass ----------------------------------------
    # Walrus codegen accepts at most one sync wait per instruction;
    # remove waits that are redundant by construction.
    out_q = {u.ant_name for u in out_dma.ins.sync_info.on_update
             if u.ant_name.startswith("DMAHW")}
    for bb in nc.main_func.blocks:
        for ins in bb.instructions:
            si = ins.sync_info
            if si is None:
                continue
            t = type(ins).__name__
            if t == "InstDrain" and len(si.on_wait) > 2:
                # Kernel-tail drain: engine completion is enforced by the
                # per-engine drains + EVSEM butterfly that follow, and
                # input-DMA completions are covered transitively by the
                # compute that consumed them.  Only the output DMA's
                # queue wait is load-bearing.
                drop = lambda w: w.ant_name not in out_q
            elif t == "InstMatmult":
                # WAW on a reused psum bank: the prior matmul's drain
                # (~128 cyc) finished >=2 matmul-streams earlier, so the
                # same-engine completion wait is dead.
                drop = lambda w: w.ant_name.startswith("PE_")
            elif getattr(ins, "engine", None) == mybir.EngineType.DVE:
                # DVE fully drains its pipe between ops; waits on earlier
                # DVE completions are satisfied at issue.
                drop = lambda w: w.ant_name.startswith("DVE_")
            elif t == "InstDMACopy":
                # Per-lane FIFO (single issuing engine per lane by
                # construction) makes own-lane waits redundant.
                own = {u.ant_name for u in si.on_update
                       if u.ant_name.startswith("DMAHW")}
                drop = lambda w: w.ant_name in own
            else:
                continue
            kept = [w for w in si.on_wait if not drop(w)]
            if len(kept) != len(si.on_wait):
                si.on_wait = kept
                ins.sync_info = si
    return nc


def _plan(lens, omax):
    """Global length-rank plan: order[r] = image of rank r; core r%8 slot
    r//8.  Group widths: Wa covers slots 0..G-1 (ranks < 64), Wb the
    rest."""
    lens = np.clip(np.asarray(lens, dtype=np.int64), 1, omax)
    order = np.argsort(lens, kind="stable")
    Wa = int(lens[order[NCORES * G - 1]])
    Wb = int(lens[order[B - 1]])
    return order, Wa, Wb


def _pack_images(x_bf, lens, order, Wa, Wb, core):
    """Build the packed, padded, D-major [D, G*(Wa+Wb)] bf16 image-object
    matrix for one core.  Slot k = image order[8k + core]; its first
    lens[i] objects, padded to the group width by replicating object 0."""
    cols = []
    for k in range(IPC):
        i = order[NCORES * k + core]
        wid = Wa if k < G else Wb
        L = min(int(lens[i]), wid)
        blk = np.empty((wid, D), dtype=x_bf.dtype)
        blk[:L] = x_bf[i, :L]
        blk[L:] = x_bf[i, 0]
        cols.append(blk)
    return np.ascontiguousarray(np.concatenate(cols, axis=0).T)


def kernel(im, im_l, s, s_l, pred, pred_l, cap_o_pred, cap_o_l, c_r_pred,
           c_r_l, trace=False, tmpdir=None):
    global LAST_RESULT, _NC, _NC_KEY
    im = np.asarray(im, dtype=np.float32)
    s = np.asarray(s, dtype=np.float32)
    pred = np.asarray(pred, dtype=np.float32)
    c_r_pred = np.asarray(c_r_pred, dtype=np.float32)
    im_l = np.asarray(im_l)
    pred_l = np.asarray(pred_l)

    order1, Wa1, Wb1 = _plan(im_l, O1)
    order2, Wa2, Wb2 = _plan(pred_l, O2)
    widths = (Wa1, Wb1, Wa2, Wb2)

    im_bf = im.astype(ml_dtypes.bfloat16)
    pred_bf = pred.astype(ml_dtypes.bfloat16)

    def dmajor16(x):
        b, w, d = x.shape
        t = np.ascontiguousarray(x.transpose(1, 0, 2).reshape(w * b, d).T)
        return t.astype(ml_dtypes.bfloat16)

    capT1 = dmajor16(s)
    capT2 = dmajor16(c_r_pred)
    rblob = np.stack([1.0 / np.asarray(s_l, dtype=np.float32),
                      1.0 / np.asarray(c_r_l, dtype=np.float32)], axis=1)

    in_maps = []
    for m in range(NCORES):
        in_maps.append({
            "capT1": capT1,
            "capT2": capT2,
            "imT1": _pack_images(im_bf, im_l, order1, Wa1, Wb1, m),
            "imT2": _pack_images(pred_bf, pred_l, order2, Wa2, Wb2, m),
            "rblob": rblob,
        })

    if _NC is None or _NC_KEY != widths:
        _NC = _build_nc(widths)
        _NC_KEY = widths
    res = run_bass_kernel_spmd(_NC, in_maps, list(range(NCORES)), trace=trace,
                               tmpdir=tmpdir)
    LAST_RESULT = res

    # Each core returns [128 caps, 32]: part-1 slots then part-2 slots,
    # already scaled by 1/caption_len.  Unpermute slots back to image
    # order and add the parts.
    scores = np.zeros((B, B), dtype=np.float32)
    for m in range(NCORES):
        tile_m = res.results[m]["scores_t"]  # [128, 32]
        idx1 = order1[np.arange(IPC) * NCORES + m]
        idx2 = order2[np.arange(IPC) * NCORES + m]
        scores[idx1, :] += tile_m[:, :IPC].T
        scores[idx2, :] += tile_m[:, IPC:].T

    # Triplet margin loss on the full (tiny) B x B matrix.
    d = np.diag(scores).copy()
    cost_s = np.maximum(MARGIN + scores - d[:, None], 0.0).astype(np.float32)
    cost_im = np.maximum(MARGIN + scores - d[None, :], 0.0).astype(np.float32)
    np.fill_diagonal(cost_s, 0.0)
    np.fill_diagonal(cost_im, 0.0)
    out = cost_s.max(axis=1).sum() + cost_im.max(axis=0).sum()
    return np.asarray(out, dtype=np.float32)


# revision 54
# speedup vs baseline: 1.0021x; 1.0021x over previous
"""Trainium2 Bass kernel for nn_ContrastiveLoss_66030827208766.

Strategy (data-parallel over images, captions replicated):
  - 8 cores, 16 images each.  Images are assigned to cores by GLOBAL
    length rank (core = rank % 8, slot = rank // 8), so every core's
    slot-k image has nearly the same valid-object count.  Only valid
    objects are shipped, padded per slot-group to a shared width: group
    A = slots 0-7 padded to Wa = len_sorted[63], group B = slots 8-15
    padded to Wb = len_sorted[127].  One program serves all cores.
  - Padding replicates object 0 (always valid), so a plain max over the
    padded block equals the masked max over valid objects.
  - All matmul operands are bf16 (PE accumulates fp32; end-to-end loss
    error ~1e-5).  Captions are replicated to every core in D-major
    layout [D, w*128 + c]: each 128-column slice is one caption word
    across all 128 captions.
  - Device per core: per caption word w, one matmul (stationary caption
    chunk [D,128], moving packed image-objects [D, C]) -> PSUM bank;
    grouped strided reduce_max over each slot's object block ->
    buf[c, w, slot]; reduce_sum over w; scale by 1/caption_len ->
    two [128 caps, 16 slots] tiles (parts sort by different keys) ->
    DRAM.
  - Host: unpermute slots of each part, add, then the (tiny) triplet
    margin loss reduction in numpy.

Codegen constraint: every TPB instruction can carry at most ONE sync
wait.  Three tactics keep us within it: (1) freshly-DMA'd tiles are
first touched by degenerate 1x1 "junk" matmuls so the real matmuls'
DMA-queue requirements are already observed by the PE; (2) buffers are
laid out so each writer hits a disjoint range (no spurious WAW chains);
(3) a post-pass strips waits that are redundant by construction
(same-engine in-order completion, per-queue DMA FIFO, barrier-covered
drain waits).
"""

import ml_dtypes
import numpy as np

import concourse.bass as bass
import concourse.mybir as mybir
from concourse import tile
from concourse.bass_utils import run_bass_kernel_spmd
from concourse.tile_rust import add_dep_helper

B = 128          # batch (images == captions)
O1, W1 = 36, 50  # part 1: im objects, s words
O2, W2 = 25, 30  # part 2: pred objects, c_r words
D = 128
NCORES = 8
IPC = B // NCORES  # images (slots) per core
G = IPC // 2       # slots per width-group
MARGIN = 0.2
F32 = mybir.dt.float32
BF16 = mybir.dt.bfloat16

LAST_RESULT = None   # BassKernelResults of the most recent run (for test.py)
_NC = None           # cached program
_NC_KEY = None       # widths the cached program was built for


def _build_part(nc, pending, hoist, cap, imt, buf, ps_tiles, ps_cols,
                W, Wa, Wb, cap_piece_cols, cap_key):
    """Emit matmul + grouped-reduce stream for one t2i part.

    Chunk layout: if C = 8*(Wa+Wb) fits one PSUM bank, chunk j of a tile
    sits in bank j (group A at +0, group B at +8*Wa); otherwise each
    chunk takes two banks (A at +0, B at +512).
    """
    C = G * (Wa + Wb)
    if C <= 512:
        banks_per_chunk, offA, offB = 1, 0, G * Wa
    else:
        banks_per_chunk, offA, offB = 2, 0, 512
    wc_per_piece = cap_piece_cols // B

    w = 0
    t_idx = 0
    while w < W:
        ps = ps_tiles[t_idx % len(ps_tiles)]
        cap_chunks = ps_cols[t_idx % len(ps_tiles)] // (512 * banks_per_chunk)
        n = min(cap_chunks, W - w)
        t_idx += 1
        for j in range(n):
            pc = (w + j) // wc_per_piece
            hoist((cap_key, pc),
                  cap[:1, pc * cap_piece_cols:pc * cap_piece_cols + 1])
            cs = cap[:, (w + j) * B:(w + j + 1) * B]
            base = j * banks_per_chunk * 512
            if banks_per_chunk == 1:
                mm = nc.tensor.matmul(ps[:, base:base + C], cs, imt[:],
                                      start=True, stop=True)
                while pending:
                    add_dep_helper(mm.ins, pending.pop().ins, sync=False,
                                   reason="order matmul after wait-carrier")
            else:
                for off, w0, wid in ((offA, 0, Wa), (offB, G * Wa, Wb)):
                    mm = nc.tensor.matmul(
                        ps[:, base + off:base + off + G * wid], cs,
                        imt[:, w0:w0 + G * wid], start=True, stop=True)
                    while pending:
                        add_dep_helper(mm.ins, pending.pop().ins, sync=False,
                                       reason="order matmul after wait-carrier")
        # Two grouped reduces (uniform width within each) covering all n
        # chunks of this tile.
        stride = banks_per_chunk * 512
        v = ps[:, :n * stride].rearrange("p (c x) -> p c x", c=n)
        for off, wid, s0, s1 in ((offA, Wa, 0, G), (offB, Wb, G, IPC)):
            nc.vector.reduce_max(
                buf[:, w:w + n, s0:s1],
                v[:, :, off:off + G * wid].rearrange(
                    "p c (g o) -> p c g o", o=wid),
                axis=mybir.AxisListType.X,
            )
        w += n


def _build_nc(widths):
    (Wa1, Wb1, Wa2, Wb2) = widths
    nc = bass.Bass()
    C1 = G * (Wa1 + Wb1)
    C2 = G * (Wa2 + Wb2)
    capT1 = nc.dram_tensor("capT1", [D, B * W1], BF16, kind="ExternalInput")
    capT2 = nc.dram_tensor("capT2", [D, B * W2], BF16, kind="ExternalInput")
    imT1 = nc.dram_tensor("imT1", [D, C1], BF16, kind="ExternalInput")
    imT2 = nc.dram_tensor("imT2", [D, C2], BF16, kind="ExternalInput")
    rblob = nc.dram_tensor("rblob", [B, 2], F32, kind="ExternalInput")
    out_t = nc.dram_tensor("scores_t", [B, 2 * IPC], F32,
                           kind="ExternalOutput")

    with tile.TileContext(nc) as tc:
        with (
            tc.tile_pool(name="const", bufs=1) as cpool,
            tc.tile_pool(name="psum", bufs=1, space="PSUM") as pspool,
            tc.tile_pool(name="work", bufs=1) as wpool,
        ):
            # ---- input DMAs: 16 total, alternating the two HWDGE rings
            # (even index -> sync, odd -> scalar).  DMAHW bookkeeping
            # lanes are assigned by global round-robin, so each of the 8
            # lanes sees a single issuing engine -> per-lane FIFO holds
            # and own-lane waits are strippable.  The output DMA is
            # emission #16 -> lane 0 (sync), same engine as lane 0's
            # inputs.
            dma_idx = [0]

            def load(dst_ap, src_ap):
                eng = nc.sync if dma_idx[0] % 2 == 0 else nc.scalar
                dma_idx[0] += 1
                return eng.dma_start(dst_ap, src_ap)

            NP1 = 10  # cap1 pieces (5 w-chunks each): early words land early
            NP2 = 1   # cap2: one DMA, only needed after part 1 finishes
            P1C = B * W1 // NP1
            P2C = B * W2 // NP2

            imt1 = cpool.tile([D, C1], BF16, tag="imt1")
            load(imt1[:], imT1[:])
            cap1 = cpool.tile([D, B * W1], BF16, tag="cap1")
            for j in range(NP1):
                load(cap1[:, j * P1C:(j + 1) * P1C],
                     capT1[:, j * P1C:(j + 1) * P1C])
            imt2 = cpool.tile([D, C2], BF16, tag="imt2")
            load(imt2[:], imT2[:])
            cap2 = cpool.tile([D, B * W2], BF16, tag="cap2")
            for j in range(NP2):
                load(cap2[:, j * P2C:(j + 1) * P2C],
                     capT2[:, j * P2C:(j + 1) * P2C])
            rblob_sb = cpool.tile([B, 2], F32, tag="rblob")
            load(rblob_sb[:], rblob[:])
            r1 = rblob_sb[:, 0:1]
            r2 = rblob_sb[:, 1:2]
            # 14 input DMAs; the output DMA is emission #14 -> lane 6,
            # whose earlier user (emission #6) is also sync-issued.
            assert dma_idx[0] == 14, dma_idx

            # w-major: each reduce writes a disjoint contiguous-ish range.
            buf1 = wpool.tile([B, W1, IPC], F32, tag="buf1")
            buf2 = wpool.tile([B, W2, IPC], F32, tag="buf2")

            # Static PSUM: 4-bank + 3-bank ping-pong tiles shared by both
            # parts, 1 junk bank.  (Pool slot rotation would bundle both
            # accessor engines' release waits onto one matmul.)
            psA = pspool.tile([B, 2048], F32, tag="psA", name="psA")
            psB = pspool.tile([B, 1536], F32, tag="psB", name="psB")
            junk_ps = pspool.tile([1, 1], F32, tag="junk_ps", name="junk_ps")

            hoisted = {}
            pending = []

            def hoist(key, corner_ap):
                if key in hoisted:
                    return
                hoisted[key] = nc.tensor.matmul(
                    junk_ps[:, :], corner_ap, corner_ap,
                    start=True, stop=True, skip_group_check=True,
                )
                pending.append(hoisted[key])

            sout = wpool.tile([B, 2 * IPC], F32, tag="sout")
            s1 = wpool.tile([B, IPC], F32, tag="s1")
            s2 = wpool.tile([B, IPC], F32, tag="s2")

            hoist(("imt1",), imt1[:1, :1])
            _build_part(nc, pending, hoist, cap1, imt1, buf1, [psA, psB],
                        [2048, 1536], W1, Wa1, Wb1, P1C, "cap1")
            # Part-1 epilogue emitted before part 2: the DVE executes its
            # queue in order, so this overlaps part-2 matmuls.
            nc.vector.reduce_sum(s1[:], buf1[:].rearrange("p w i -> p i w"),
                                 axis=mybir.AxisListType.X)
            nc.vector.tensor_scalar_mul(sout[:, :IPC], s1[:], r1)

            hoist(("imt2",), imt2[:1, :1])
            _build_part(nc, pending, hoist, cap2, imt2, buf2, [psA, psB],
                        [2048, 1536], W2, Wa2, Wb2, P2C, "cap2")
            nc.vector.reduce_sum(s2[:], buf2[:].rearrange("p w i -> p i w"),
                                 axis=mybir.AxisListType.X)
            nc.vector.tensor_scalar_mul(sout[:, IPC:], s2[:], r2)
            out_dma = nc.sync.dma_start(out_t[:], sout[:])

    # ---- wait-strip post-pass ----------------------------------------
    # Walrus codegen accepts at most one sync wait per instruction;
    # remove waits that are redundant by construction.
    out_q = {u.ant_name for u in out_dma.ins.sync_info.on_update
             if u.ant_name.startswith("DMAHW")}
    for bb in nc.main_func.blocks:
        for ins in bb.instructions:
            si = ins.sync_info
            if si is None:
                continue
            t = type(ins).__name__
            if t == "InstDrain" and len(si.on_wait) > 2:
                # Kernel-tail drain: engine completion is enforced by the
                # per-engine drains + EVSEM butterfly that follow, and
                # input-DMA completions are covered transitively by the
                # compute that consumed them.  Only the output DMA's
                # queue wait is load-bearing.
                drop = lambda w: w.ant_name not in out_q
            elif t == "InstMatmult":
                # WAW on a reused psum bank: the prior matmul's drain
                # (~128 cyc) finished >=2 matmul-streams earlier, so the
                # same-engine completion wait is dead.
                drop = lambda w: w.ant_name.startswith("PE_")
            elif getattr(ins, "engine", None) == mybir.EngineType.DVE:
                # DVE fully drains its pipe between ops; waits on earlier
                # DVE completions are satisfied at issue.
                drop = lambda w: w.ant_name.startswith("DVE_")
            elif t == "InstDMACopy":
                # Per-lane FIFO (single issuing engine per lane by
                # construction) makes own-lane waits redundant.
                own = {u.ant_name for u in si.on_update
                       if u.ant_name.startswith("DMAHW")}
                drop = lambda w: w.ant_name in own
            else:
                continue
            kept = [w for w in si.on_wait if not drop(w)]
            if len(kept) != len(si.on_wait):
                si.on_wait = kept
                ins.sync_info = si
    return nc


def _plan(lens, omax):
    """Global length-rank plan: order[r] = image of rank r; core r%8 slot
    r//8.  Group widths: Wa covers slots 0..G-1 (ranks < 64), Wb the
    rest."""
    lens = np.clip(np.asarray(lens, dtype=np.int64), 1, omax)
    order = np.argsort(lens, kind="stable")
    Wa = int(lens[order[NCORES * G - 1]])
    Wb = int(lens[order[B - 1]])
    return order, Wa, Wb


def _pack_images(x_bf, lens, order, Wa, Wb, core):
    """Build the packed, padded, D-major [D, G*(Wa+Wb)] bf16 image-object
    matrix for one core.  Slot k = image order[8k + core]; its first
    lens[i] objects, padded to the group width by replicating object 0."""
    cols = []
    for k in range(IPC):
        i = order[NCORES * k + core]
        wid = Wa if k < G else Wb
        L = min(int(lens[i]), wid)
        blk = np.empty((wid, D), dtype=x_bf.dtype)
        blk[:L] = x_bf[i, :L]
        blk[L:] = x_bf[i, 0]
        cols.append(blk)
    return np.ascontiguousarray(np.concatenate(cols, axis=0).T)


def kernel(im, im_l, s, s_l, pred, pred_l, cap_o_pred, cap_o_l, c_r_pred,
           c_r_l, trace=False, tmpdir=None):
    global LAST_RESULT, _NC, _NC_KEY
    im = np.asarray(im, dtype=np.float32)
    s = np.asarray(s, dtype=np.float32)
    pred = np.asarray(pred, dtype=np.float32)
    c_r_pred = np.asarray(c_r_pred, dtype=np.float32)
    im_l = np.asarray(im_l)
    pred_l = np.asarray(pred_l)

    order1, Wa1, Wb1 = _plan(im_l, O1)
    order2, Wa2, Wb2 = _plan(pred_l, O2)
    widths = (Wa1, Wb1, Wa2, Wb2)

    im_bf = im.astype(ml_dtypes.bfloat16)
    pred_bf = pred.astype(ml_dtypes.bfloat16)

    def dmajor16(x):
        b, w, d = x.shape
        t = np.ascontiguousarray(x.transpose(1, 0, 2).reshape(w * b, d).T)
        return t.astype(ml_dtypes.bfloat16)

    capT1 = dmajor16(s)
    capT2 = dmajor16(c_r_pred)
    rblob = np.stack([1.0 / np.asarray(s_l, dtype=np.float32),
                      1.0 / np.asarray(c_r_l, dtype=np.float32)], axis=1)

    in_maps = []
    for m in range(NCORES):
        in_maps.append({
            "capT1": capT1,
            "capT2": capT2,
            "imT1": _pack_images(im_bf, im_l, order1, Wa1, Wb1, m),
            "imT2": _pack_images(pred_bf, pred_l, order2, Wa2, Wb2, m),
            "rblob": rblob,
        })

    if _NC is None or _NC_KEY != widths:
        _NC = _build_nc(widths)
        _NC_KEY = widths
    res = run_bass_kernel_spmd(_NC, in_maps, list(range(NCORES)), trace=trace,
                               tmpdir=tmpdir)
    LAST_RESULT = res

    # Each core returns [128 caps, 32]: part-1 slots then part-2 slots,
    # already scaled by 1/caption_len.  Unpermute slots back to image
    # order and add the parts.
    scores = np.zeros((B, B), dtype=np.float32)
    for m in range(NCORES):
        tile_m = res.results[m]["scores_t"]  # [128, 32]
        idx1 = order1[np.arange(IPC) * NCORES + m]
        idx2 = order2[np.arange(IPC) * NCORES + m]
        scores[idx1, :] += tile_m[:, :IPC].T
        scores[idx2, :] += tile_m[:, IPC:].T

    # Triplet margin loss on the full (tiny) B x B matrix.
    d = np.diag(scores).copy()
    cost_s = np.maximum(MARGIN + scores - d[:, None], 0.0).astype(np.float32)
    cost_im = np.maximum(MARGIN + scores - d[None, :], 0.0).astype(np.float32)
    np.fill_diagonal(cost_s, 0.0)
    np.fill_diagonal(cost_im, 0.0)
    out = cost_s.max(axis=1).sum() + cost_im.max(axis=0).sum()
    return np.asarray(out, dtype=np.float32)
